# revision 1
# baseline (speedup 1.0000x reference)
"""Trainium2 Bass kernel for additive (tanh) attention with mask.

Computation (per batch b):
    wah    = h @ W_ah.T                             [B, H]
    e      = tanh(wah[:, None, :] + p_att_feats)    [B, M, H]
    logits = e @ w_alpha                            [B, M]
    logits = where(mask == 0, -1e9, logits)
    alpha  = softmax(logits, -1)
    att    = alpha @ att_feats                      [B, D]

Strategy: pure data-parallel over batch (8 batches / core on 8 cores).
Masked rows contribute exactly 0 to the softmax-weighted sum, so the
kernel only streams the ~50% of att_feats / p_att_feats rows with
mask==1, gathered by row index with SWDGE dma_gather, and ships both
bulk streams as bf16 (~21 MB/core vs 80 MB dense fp32).  exp() is
applied without max-subtraction (logits are bounded:
|logits| <= ||w_alpha||_1 with e in [-1,1]), masked/pad rows get an
additive -1e9 bias so their exp underflows to exactly 0, and the
normalization by 1/sum is applied once at PSUM drain time.  The
weighted-sum matmuls run bf16 x bf16 -> fp32 PSUM; exp() writes its
bf16 PE-weight tile directly (no cast pass).

Host-side work is limited to marshalling: batch->core assignment
(balanced by mask count so the SPMD gather sizes match across cores),
mask->row-index/bias tables, dtype/layout permutations of the inputs
(the wah matmul itself runs on device).

Measured on 8xNC-v3 (axon): ~85 us/core per full pass (paired
interleaved For_i-slope method, n2=208 so the slope signal dominates
the tens-of-ms axon dispatch jitter).  Decomposition measured with
gather-only / compute-only builds: DMA-only ~66 us (vs the 60.3 us
21.7MB/360B/ns line-rate floor), compute-only ~60 us, full ~85 us.
Knob scan results (paired within-run A/B): fsplit=4 + 2-slot gather
prefetch + fbufs=4 is worth ~3-5 us over the old fsplit=2/pf=1/bufs=3;
multi-queue SWDGE (up to 4 hw queues) is ~2-3 us WORSE (one queue
already drains through all 16 SDMA engines); multi-row gather
descriptors over mask runs (10-15KB elem_size with elem_step=CW
overlap) are 15-26 us WORSE despite 32% fewer descriptors -- big
single-packet descriptors transfer slower per byte on real SDMA.
Compute-side 60 us is insensitive to DVE dtype width (bf16
intermediates), instruction order (all-adds-first software
pipelining), Act dispatch batching (one exp per slot), PSUM
triple-buffering via the 0/32/64 base partitions, and moving the
softmax tree to gpsimd -- all flat or worse, consistent with a shared
SBUF-port bandwidth wall rather than any engine's issue rate.
Numerics vs fp32 reference: rel-err ~2.5e-3, absmax-relative ~2.3e-3
(bf16 input quantization; f32r variants give 2.4e-4 at ~1.6x the time
via KERNEL_ATT_DTYPE/KERNEL_P_DTYPE).

Implementation notes (hard-won):
  - InstTensorTensorReduce crashes the NRT exec on this runtime; the
    logits dot-product uses the fused scalar_tensor_tensor (+accum
    row-sum), which is fine on HW.
  - float32r matmul operands must be *produced* as float32r (BIR
    verifier); DRAM tensors are declared f32r/bf16 and exp() writes the
    PE-weight tile in that dtype directly.
  - Cross-partition reduction for the softmax denominator is a DVE-only
    copy/add log-tree + 32x32 stream transpose; gpsimd
    partition_all_reduce would contend with gather descriptor
    generation on the Pool engine.
  - Phase-1 SBUF pools are opened before the phase-0 scratch pool so
    the stack allocator gives them non-overlapping addresses (otherwise
    a false overlap-dependency stalls the first gathers ~17 us).
  - wah row broadcast to 128 partitions uses a one-hot lhsT matmul
    (oh_j.T @ wah) -- no SBUF->SBUF DMA on the critical path.

Self-contained: hardcodes B=64, M=1024, RNN=1024, H=512, D=2048, 8 cores.
"""

import os

import numpy as np

import concourse.bacc as bacc
import concourse.bass as bass
import concourse.mybir as mybir
from concourse import bass_isa, library_config
from concourse.bass_utils import run_bass_kernel_spmd
from concourse.tile import TileContext

B, M, RNN, H, D = 64, 1024, 1024, 512, 2048
NCORES = 8
BL = B // NCORES  # batches per core
NEG = -1e9
F32 = mybir.dt.float32
F32R = mybir.dt.float32r
I16 = mybir.dt.int16

# Dtype of the gathered att_feats stream + PE weighted-sum matmul:
#   bf16 (default): halves the dominant DMA stream; output err ~1e-3
#   f32r: full 4-byte stream, tf32-like matmul; output err ~2e-4
#   f32:  full precision, but the PE runs at 1/4 rate
ATT_DT = os.environ.get("KERNEL_ATT_DTYPE", "bf16")
ATT_FP32 = ATT_DT == "f32"
# Dtype of the gathered p_att_feats stream (tanh input)
P_DT = os.environ.get("KERNEL_P_DTYPE", "bf16")


def _plan(mask: np.ndarray):
    """Assign batches to (core, slot) balanced by unmasked count; compute
    per-slot padded gather sizes (identical across cores - SPMD)."""
    n = mask.sum(axis=1).astype(np.int64)  # [B]
    order = np.argsort(-n, kind="stable")
    batch_of = np.empty((NCORES, BL), dtype=np.int64)
    for j in range(BL):
        for c in range(NCORES):
            batch_of[c, j] = order[j * NCORES + c]
    nbar = np.empty(BL, dtype=np.int64)
    for j in range(BL):
        mx = max(int(n[batch_of[c, j]]) for c in range(NCORES))
        nbar[j] = ((mx + 15) // 16) * 16  # multiple of 16 for idx wrap
    nch = [(int(v) + 127) // 128 for v in nbar]
    return batch_of, n, nbar, nch


SEG_LS = (2, 1)  # bucket lengths, descending


def _segments(row, LS=SEG_LS):
    """Greedy decomposition of a 0/1 row into consecutive-run segments of
    the lengths in LS (descending).  Returns {L: [start, ...]}."""
    out = {L: [] for L in LS}
    Mlen = row.shape[0]
    i = 0
    while i < Mlen:
        if not row[i]:
            i += 1
            continue
        j = i
        while j < Mlen and row[j]:
            j += 1
        ln = j - i
        p = i
        for L in LS:
            while ln >= L:
                out[L].append(p)
                p += L
                ln -= L
        i = j
    return out


def _plan_seg(mask: np.ndarray, LS=SEG_LS):
    """Segment-bucketed plan: batches -> (core, slot) minimizing the
    cross-core spread of per-bucket segment counts (the spread is padded
    with row-0 fetches so every core runs the identical program)."""
    n = mask.sum(axis=1).astype(np.int64)
    segs = [_segments(mask[b], LS) for b in range(mask.shape[0])]
    cnt = {L: np.array([len(segs[b][L]) for b in range(mask.shape[0])])
           for L in LS}
    # start from the by-n assignment, then greedy-swap to minimize
    # sum_j sum_L max_c cnt  (== rows fetched incl. padding)
    order = list(np.argsort(-n, kind="stable"))
    groups = [order[j * NCORES : (j + 1) * NCORES] for j in range(BL)]

    def slot_cost(g):
        return sum(max(int(cnt[L][b]) for b in g) * L for L in LS)

    cost = [slot_cost(g) for g in groups]
    improved = True
    while improved:
        improved = False
        for ja in range(BL):
            for jb in range(ja + 1, BL):
                for ia in range(NCORES):
                    for ib in range(NCORES):
                        ga = groups[ja][:]
                        gb = groups[jb][:]
                        ga[ia], gb[ib] = gb[ib], ga[ia]
                        ca, cb = slot_cost(ga), slot_cost(gb)
                        if ca + cb < cost[ja] + cost[jb]:
                            groups[ja], groups[jb] = ga, gb
                            cost[ja], cost[jb] = ca, cb
                            improved = True
    batch_of = np.array(groups, dtype=np.int64).T  # [NCORES, BL]
    vcnt = {L: np.array([max(int(cnt[L][b]) for b in groups[j])
                         for j in range(BL)]) for L in LS}
    n16 = {L: (vcnt[L] + 15) // 16 * 16 for L in LS}
    nchk = {L: (n16[L] + 127) // 128 for L in LS}
    return dict(batch_of=batch_of, segs=segs, cnt=cnt, vcnt=vcnt,
                n16=n16, nchk=nchk, LS=LS)


def _build(nbar, nch, reps=1, bench_mode=False, loop_n=0, fsplit=2,
           ring=32768, fbufs=3, spkt=False, nq=1, qmode="slot", pf=1,
           seg=None, nocompute=False, nodma=False, tree="dve", lt=False,
           e16=False, bexp=False, sw2=False, bslot=False, mmw=512,
           ps8=False):
    """Build the SPMD bass program (same for all cores).  reps>1 repeats
    phase 1 (benchmark amplification only; outputs are overwritten).
    bench_mode replaces the two bulk inputs (feats/p) with device-side
    zero-filled internal DRAM so per-call host transfer is tiny.
    nq = number of SWDGE queues (1..4); qmode picks the gather->queue
    assignment ("slot": per batch slot, "piece": per split piece).
    seg = segment-bucketed plan from _plan_seg (multi-row descriptors
    over consecutive unmasked runs); None = one-row-per-descriptor."""
    if seg is not None:
        LS = seg["LS"]
        vcnt, n16, nchk = seg["vcnt"], seg["n16"], seg["nchk"]
        soff_seg = {}
        off = 0
        for j in range(BL):
            for L in LS:
                soff_seg[(j, L)] = off
                off += int(n16[L][j]) // 16
        stot = off
        ncol_j = [int(sum(int(nchk[L][j]) * L for L in LS)) for j in range(BL)]
        boff = np.cumsum([0] + ncol_j)
        tch = int(boff[-1])
        max_ncol = max(ncol_j)
        nchmax = {L: int(max(nchk[L])) for L in LS}
    else:
        stot = int(sum(v // 16 for v in nbar))  # idx columns (int16)
        tch = int(sum(nch))  # total chunks (bias columns)
        soff = np.cumsum([0] + [int(v) // 16 for v in nbar])
        boff = np.cumsum([0] + list(nch))
        max_nch = max(nch)
        max_ncol = max_nch

    FATT = {"bf16": mybir.dt.bfloat16, "f32r": F32R, "f32": F32}[ATT_DT]
    nc = bacc.Bacc(
        "TRN2", target_bir_lowering=False, dynamic_dma_scratch_size=ring,
        num_swdge_queues=nq,
    )
    # p and feats are host-concatenated row-wise into one tensor so each
    # unmasked row is ONE large gather descriptor (5KB) instead of a 1KB +
    # a 4KB one -- real SDMA throughput is descriptor-overhead sensitive.
    assert ATT_DT == P_DT or ATT_FP32 == (P_DT != "bf16")
    CW = H + D  # combined row width (elements)
    if bench_mode:
        comb_d = nc.dram_tensor("comb_i", [BL * M, CW], FATT)
    else:
        comb_d = nc.dram_tensor("comb", [BL * M, CW], FATT, kind="ExternalInput")
    # W^T and h^T arrive pre-permuted from the host (layout marshalling):
    # wt[p, rc, hh] = W[hh, rc*128+p], ht[p, rc, b] = h[b, rc*128+p].
    # f32r dram views let the PE consume them at 1 cycle/row.
    wt_d = nc.dram_tensor("wt", [128, RNN // 128, H], F32R, kind="ExternalInput")
    ht_d = nc.dram_tensor("ht", [128, RNN // 128, BL], F32R, kind="ExternalInput")
    wa_d = nc.dram_tensor("walpha", [1, H], F32R, kind="ExternalInput")
    # oh[b, j*128+p] = (b == j): one-hot lhsT used to broadcast row j of the
    # [BL, H] wah tile to all 128 partitions without any SBUF->SBUF move
    oh_d = nc.dram_tensor("oh", [BL, BL * 128], F32R, kind="ExternalInput")
    idx_d = nc.dram_tensor("idx", [128, stot], I16, kind="ExternalInput")
    bias_d = nc.dram_tensor("bias", [128, tch], F32, kind="ExternalInput")
    ones_d = nc.dram_tensor("ones", [1, 128], F32R, kind="ExternalInput")
    out_d = nc.dram_tensor("out", [BL, D], F32, kind="ExternalOutput")

    RC = RNN // 128  # 8

    import contextlib

    with TileContext(nc) as tc:
        nc.gpsimd.load_library(library_config.mlp)
        # Pool order matters: phase-1 pools (fp/pp/wk/sm) are allocated
        # BEFORE the phase-0 scratch pool so their SBUF addresses do not
        # overlap it -- otherwise the stack allocator's overlap-dep would
        # stall the first gathers until all of phase 0 has drained.
        with contextlib.ExitStack() as stk:
            cp = stk.enter_context(tc.tile_pool(name="const", bufs=1))
            if seg is not None:
                fpL = {
                    L: stk.enter_context(
                        tc.tile_pool(name=f"fp{L}", bufs=fbufs)
                    )
                    for L in LS
                }
            else:
                fp = stk.enter_context(tc.tile_pool(name="fp", bufs=fbufs))
            lp = stk.enter_context(tc.tile_pool(name="lp", bufs=4))
            wk = stk.enter_context(
                tc.tile_pool(name="wk", bufs=(8 if sw2 else 4))
            )
            sm = stk.enter_context(tc.tile_pool(name="sm", bufs=3))
            op = stk.enter_context(tc.tile_pool(name="op", bufs=2))
            idx_t = cp.tile([128, stot], I16)
            nc.sync.dma_start(idx_t[:, :], idx_d[:, :])
            if bench_mode:
                # zero-fill the internal bulk tensor once (phase -1)
                with tc.tile_pool(name="fill", bufs=1) as fillp:
                    ztf = fillp.tile([128, CW], FATT)
                    nc.vector.memset(ztf[:, :], 0.0)
                    for blk in range(BL * M // 128):
                        nc.sync.dma_start(
                            comb_d[blk * 128 : (blk + 1) * 128, :], ztf[:, :]
                        )
            bias_t = cp.tile([128, tch], F32)
            nc.sync.dma_start(bias_t[:, :], bias_d[:, :])
            wahb = cp.tile([128, BL, H], F32)  # per-slot wah broadcast
            walphab = cp.tile([128, H], F32)  # w_alpha broadcast

            # ---------------- phase 0: wah = h @ W.T, broadcasts ----------
            with (
                tc.tile_pool(name="ph0", bufs=1) as p0,
                tc.tile_pool(name="ph0w", bufs=2) as p0w,
                tc.tile_pool(name="ph0ps", bufs=2, space="PSUM") as p0ps,
            ):
                ones_sb = p0.tile([1, 128], F32R)
                nc.sync.dma_start(ones_sb[:, :], ones_d[:, :])
                oh_sb = p0.tile([BL, BL * 128], F32R)
                nc.sync.dma_start(oh_sb[:, :], oh_d[:, :])
                wa_sb = p0.tile([1, H], F32R)
                nc.sync.dma_start(wa_sb[:, :], wa_d[:, :])
                wt_sb = p0.tile([128, RC, H], F32R)
                nc.sync.dma_start(wt_sb[:, :, :], wt_d[:, :, :])
                ht_sb = p0.tile([128, RC, BL], F32R)
                nc.sync.dma_start(ht_sb[:, :, :], ht_d[:, :, :])

                # wah [b, h] = sum_r h^T.T @ W^T
                ps_wah = p0ps.tile([BL, H], F32, tag="wah")
                for rc in range(RC):
                    nc.tensor.matmul(
                        ps_wah[:, :],
                        ht_sb[:, rc, :],
                        wt_sb[:, rc, :],
                        start=(rc == 0),
                        stop=(rc == RC - 1),
                    )
                wah_sb = p0.tile([BL, H], F32R)
                nc.vector.tensor_copy(wah_sb[:, :], ps_wah[:, :])
                # broadcast row j to 128 partitions: onehot_j.T @ wah_sb
                for j in range(BL):
                    pb = p0ps.tile([128, H], F32, tag="bc")
                    nc.tensor.matmul(
                        pb[:, :],
                        oh_sb[:, j * 128 : (j + 1) * 128],
                        wah_sb[:, :],
                        start=True, stop=True,
                    )
                    nc.scalar.copy(wahb[:, j, :], pb[:, :])
                pb = p0ps.tile([128, H], F32, tag="bc")
                nc.tensor.matmul(
                    pb[:, :], ones_sb[:, :], wa_sb[:, :], start=True, stop=True
                )
                nc.scalar.copy(walphab[:, :], pb[:, :])

            # ---------------- phase 1: per-slot sparse attention ----------
            if seg is not None:
                # multi-row-descriptor sources: row stride CW, element size
                # L*CW (overlapping view; a segment descriptor fetches L
                # consecutive rows of comb in one burst)
                base_ap = comb_d[:, :]
                src_of = {
                    L: (
                        base_ap
                        if L == 1
                        else bass.AP(
                            base_ap.tensor,
                            base_ap.offset,
                            [(CW, BL * M - (L - 1)), (1, L * CW)],
                        )
                    )
                    for L in LS
                }

                def f_shape(L):
                    return [128, nchmax[L], L * CW]

                # first-pass garbage guard: -1-padded segment slots are
                # never DMA-written; tanh must not see uninitialized SBUF
                for _ in range(fbufs):
                    for L in LS:
                        tz = fpL[L].tile(f_shape(L), FATT, tag=f"f{L}")
                        for cc in range(nchmax[L]):
                            nc.vector.memset(tz[:, cc, :], 0.0)

                def issue_f_gather(j):
                    tiles = {
                        L: fpL[L].tile(
                            f_shape(L), FATT, tag=f"f{L}", name=f"f{L}"
                        )
                        for L in LS
                    }
                    for L in LS if not nodma else ():
                        v, m16 = int(vcnt[L][j]), int(n16[L][j])
                        cj = int(nchk[L][j])
                        if m16 == 0:
                            continue
                        s0 = soff_seg[(j, L)]
                        per = max(1, (cj + fsplit - 1) // fsplit)
                        c0 = 0
                        while c0 < cj:
                            c1 = min(cj, c0 + per)
                            r0, r1 = c0 * 128, min(m16, c1 * 128)
                            nvalid = min(v, r1) - r0
                            nc.gpsimd.dma_gather(
                                tiles[L][:, c0:c1, :], src_of[L],
                                idx_t[:, s0 + r0 // 16 : s0 + r1 // 16],
                                r1 - r0, nvalid, L * CW,
                                elem_step=(CW if L > 1 else None),
                                single_packet=spkt,
                            )
                            c0 = c1
                    return tiles

                def cols_of(j, tiles):
                    cols = []
                    for L in LS:
                        for c in range(int(nchk[L][j])):
                            kc = min(128, int(vcnt[L][j]) - c * 128)
                            for k in range(L):
                                cols.append((tiles[L], c, k * CW, kc))
                    return cols

            else:

                def issue_f_gather(j):
                    nj, cj = int(nbar[j]), nch[j]
                    f_t = fp.tile([128, max_nch, CW], FATT, tag="f")
                    # split the gather so the pipeline starts on the first
                    # piece while the rest streams
                    s0 = int(soff[j])
                    per = max(1, (cj + fsplit - 1) // fsplit)
                    c0 = cj if nodma else 0
                    if nodma:  # allocate the tile so compute may read it
                        nc.vector.memset(f_t[:, 0, 0:16], 0.0)
                    piece = 0
                    while c0 < cj:
                        c1 = min(cj, c0 + per)
                        r0, r1 = c0 * 128, min(nj, c1 * 128)
                        if qmode == "piece":
                            q = (j * fsplit + piece) % nq
                        else:
                            q = j % nq
                        nc.gpsimd.dma_gather(
                            f_t[:, c0:c1, :], comb_d[:, :],
                            idx_t[:, s0 + r0 // 16 : s0 + r1 // 16],
                            r1 - r0, r1 - r0, CW, single_packet=spkt,
                            queue_num=q,
                        )
                        c0 = c1
                        piece += 1
                    return {1: f_t}

                def cols_of(j, tiles):
                    nj, cj = int(nbar[j]), nch[j]
                    return [
                        (tiles[1], c, 0, min(128, nj - c * 128))
                        for c in range(cj)
                    ]

                if bslot:
                    # batched-slot compute touches all 128 partitions of
                    # every chunk; memset once so no pass ever tanh's
                    # uninitialized SBUF (NaN would poison the PSUM)
                    for _ in range(fbufs):
                        tz = fp.tile([128, max_nch, CW], FATT, tag="f")
                        for cc in range(max_nch):
                            nc.vector.memset(tz[:, cc, :], 0.0)

            EDT = mybir.dt.bfloat16 if e16 else F32
            assert not (bslot and seg is not None)
            if bslot:
                bb = stk.enter_context(tc.tile_pool(name="bb", bufs=3))

            with tc.tile_pool(
                name="aps", bufs=(1 if ps8 else 2), space="PSUM"
            ) as aps:
                if ps8:
                    # one persistent accumulator; slot j owns PSUM partition
                    # 32*(j%3) (matmul out base must be 0/32/64), giving
                    # 3-way buffering -- the 2-buffer pool serialized j+1's
                    # matmuls behind j-1's drain
                    pstile = aps.tile([65, D], F32, name="pst")

                def emit_tail(j, exr, ncols, ps):
                    # softmax denominator: free-dim reduce, then a partition
                    # reduction (gpsimd all-reduce or DVE-only log-tree),
                    # then scale the PSUM accumulator by 1/s at drain time
                    rowsum = sm.tile([128, 1], F32, tag="rs")
                    nc.vector.tensor_reduce(
                        rowsum[:, :],
                        exr[:, :ncols],
                        axis=mybir.AxisListType.X,
                        op=mybir.AluOpType.add,
                    )
                    if tree == "pool":
                        allr = sm.tile([128, 1], F32, tag="ar")
                        nc.gpsimd.partition_all_reduce(
                            allr[:, :], rowsum[:, :], 128,
                            bass_isa.ReduceOp.add,
                        )
                        sv = allr
                    else:
                        c1 = sm.tile([64, 1], F32, tag="c1")
                        nc.vector.tensor_copy(c1[:, :], rowsum[64:128, :])
                        a1 = sm.tile([64, 1], F32, tag="a1")
                        nc.vector.tensor_add(a1[:, :], rowsum[0:64, :], c1[:, :])
                        c2 = sm.tile([32, 1], F32, tag="c2")
                        nc.vector.tensor_copy(c2[:, :], a1[32:64, :])
                        stg = sm.tile([32, 32], F32, tag="stg")
                        nc.vector.memset(stg[:, :], 0.0)
                        nc.vector.tensor_add(stg[:, 0:1], a1[0:32, :], c2[:, :])
                        trp = sm.tile([32, 32], F32, tag="trp")
                        nc.vector.transpose(trp[:, :], stg[:, :])
                        sv = sm.tile([1, 1], F32, tag="sv")
                        nc.vector.tensor_reduce(
                            sv[0:1, :],
                            trp[0:1, :],
                            axis=mybir.AxisListType.X,
                            op=mybir.AluOpType.add,
                        )
                    rinv = sm.tile([1, 1], F32, tag="ri")
                    nc.vector.reciprocal(rinv[:, :], sv[0:1, :])
                    att = op.tile([1, D], F32, tag="at")
                    nc.scalar.activation(
                        att[:, :],
                        ps[0:1, :],
                        mybir.ActivationFunctionType.Copy,
                        scale=rinv[0:1, :],
                    )
                    nc.sync.dma_start(out_d[j : j + 1, :], att[:, :])

                loop_cm = (
                    tc.For_i(0, loop_n, 1,
                             hint_engines=tuple(mybir.ALL_ENGINES))
                    if loop_n else contextlib.nullcontext()
                )
                with loop_cm:
                  for rep in range(reps):
                    pending = [issue_f_gather(jj) for jj in range(min(pf, BL))]
                    pend_tail = None
                    for j in range(BL):
                        f_tiles = pending.pop(0)
                        if j + pf < BL:
                            pending.append(issue_f_gather(j + pf))
                        if nocompute:
                            continue
                        cols = cols_of(j, f_tiles)
                        ncols = len(cols)

                        logits = lp.tile([128, max_ncol], F32, tag="lg")
                        if not bslot:
                            nc.vector.memset(logits[:, :], 0.0)
                        exr = lp.tile([128, max_ncol], FATT, tag="exr")
                        if ps8:
                            pb_ = 32 * (j % 3)
                            ps = pstile[pb_ : pb_ + 1, :]
                        else:
                            ps = aps.tile([1, D], F32, tag="att")
                        if bslot:
                            # whole-slot batched chain: one instruction per
                            # op over a [128, cj, H] strided view; broadcast
                            # APs supply wah/w_alpha along the chunk dim
                            f_t = cols[0][0]
                            cj = ncols
                            eb = bb.tile([128, cj, H], EDT, tag="eb")
                            wah_b = wahb[:, j, :].unsqueeze(1).broadcast_to(
                                [128, cj, H]
                            )
                            nc.vector.tensor_add(
                                eb[:, :, :], f_t[:, 0:cj, 0:H], wah_b
                            )
                            nc.scalar.activation(
                                eb[:, :, :], eb[:, :, :],
                                mybir.ActivationFunctionType.Tanh,
                            )
                            wal_b = walphab[:, :].unsqueeze(1).broadcast_to(
                                [128, cj, H]
                            )
                            nc.vector.scalar_tensor_tensor(
                                out=eb[:, :, :],
                                in0=eb[:, :, :],
                                scalar=1.0,
                                in1=wal_b,
                                op0=mybir.AluOpType.mult,
                                op1=mybir.AluOpType.mult,
                            )
                            nc.vector.tensor_reduce(
                                logits[:, 0:cj], eb[:, :, :],
                                axis=mybir.AxisListType.X,
                                op=mybir.AluOpType.add,
                            )
                            nc.vector.tensor_add(
                                logits[:, :cj], logits[:, :cj],
                                bias_t[:, int(boff[j]) : int(boff[j]) + cj],
                            )
                            nc.scalar.activation(
                                exr[:, :cj], logits[:, :cj],
                                mybir.ActivationFunctionType.Exp,
                            )
                            for ci, (f_t, c, koff, kc) in enumerate(cols):
                                for d in range(D // mmw):
                                    nc.tensor.matmul(
                                        ps[0:1, d * mmw : (d + 1) * mmw],
                                        exr[:kc, ci : ci + 1],
                                        f_t[:kc, c, koff + H + d * mmw : koff + H + (d + 1) * mmw],
                                        start=(ci == 0),
                                        stop=(ci == ncols - 1),
                                    )
                            if lt:
                                if pend_tail is not None:
                                    emit_tail(*pend_tail)
                                pend_tail = (j, exr, ncols, ps)
                            else:
                                emit_tail(j, exr, ncols, ps)
                            continue
                        es = []
                        if sw2:
                            # emit ALL adds first: the in-order DVE queue
                            # then never stalls waiting for a tanh between
                            # add(c) and stt(c) of the same column
                            for ci, (f_t, c, koff, kc) in enumerate(cols):
                                e = wk.tile([128, H], EDT, tag="e")
                                nc.vector.tensor_add(
                                    e[:kc, :], f_t[:kc, c, koff : koff + H],
                                    wahb[:kc, j, :]
                                )
                                es.append(e)
                        for ci, (f_t, c, koff, kc) in enumerate(cols):
                            if sw2:
                                e = es[ci]
                            else:
                                e = wk.tile([128, H], EDT, tag="e")
                                nc.vector.tensor_add(
                                    e[:kc, :], f_t[:kc, c, koff : koff + H],
                                    wahb[:kc, j, :]
                                )
                            nc.scalar.activation(
                                e[:kc, :], e[:kc, :], mybir.ActivationFunctionType.Tanh
                            )
                            # NOTE: InstTensorTensorReduce crashes the device
                            # (NRT exec error) on this runtime; the fused
                            # scalar_tensor_tensor (+accum row-sum) is fine.
                            tt = lp.tile([128, H], EDT, tag="tt")
                            nc.vector.scalar_tensor_tensor(
                                out=tt[:kc, :],
                                in0=e[:kc, :],
                                scalar=1.0,
                                in1=walphab[:kc, :],
                                op0=mybir.AluOpType.mult,
                                op1=mybir.AluOpType.mult,
                                accum_out=logits[:kc, ci : ci + 1],
                            )
                            if not bexp:
                                # exp(logits + bias); bias = -1e9 on masked/
                                # pad rows.  Output dtype doubles as the PE
                                # weight dtype (bf16/f32r) -- no cast pass.
                                nc.scalar.activation(
                                    exr[:, ci : ci + 1],
                                    logits[:, ci : ci + 1],
                                    mybir.ActivationFunctionType.Exp,
                                    bias=bias_t[:, int(boff[j]) + ci : int(boff[j]) + ci + 1],
                                )
                                lhsT = exr[:kc, ci : ci + 1]
                                for d in range(D // mmw):
                                    nc.tensor.matmul(
                                        ps[0:1, d * mmw : (d + 1) * mmw],
                                        lhsT,
                                        f_t[:kc, c, koff + H + d * mmw : koff + H + (d + 1) * mmw],
                                        start=(ci == 0),
                                        stop=(ci == ncols - 1),
                                    )
                        if bexp:
                            # batched mask-bias + single exp for the whole
                            # slot: fewer Act dispatches, and the per-column
                            # DVE->Act->PE ping-pong collapses to one hop
                            nc.vector.tensor_add(
                                logits[:, :ncols], logits[:, :ncols],
                                bias_t[:, int(boff[j]) : int(boff[j]) + ncols],
                            )
                            nc.scalar.activation(
                                exr[:, :ncols], logits[:, :ncols],
                                mybir.ActivationFunctionType.Exp,
                            )
                            for ci, (f_t, c, koff, kc) in enumerate(cols):
                                for d in range(D // mmw):
                                    nc.tensor.matmul(
                                        ps[0:1, d * mmw : (d + 1) * mmw],
                                        exr[:kc, ci : ci + 1],
                                        f_t[:kc, c, koff + H + d * mmw : koff + H + (d + 1) * mmw],
                                        start=(ci == 0),
                                        stop=(ci == ncols - 1),
                                    )
                        # emit the previous slot's tail AFTER this slot's
                        # columns: the in-order DVE/Act queues then never
                        # stall on the tail's cross-engine latency chain
                        if lt:
                            if pend_tail is not None:
                                emit_tail(*pend_tail)
                            pend_tail = (j, exr, ncols, ps)
                        else:
                            emit_tail(j, exr, ncols, ps)
                    if pend_tail is not None:
                        emit_tail(*pend_tail)
    nc.compile()
    return nc


_CACHE: dict = {}

# mode + build knobs used for the real kernel AND the hw bench (test.py);
# updated as A/B experiments conclude
MODE = os.environ.get("KERNEL_MODE", "rows")
BEST = dict(fsplit=4, pf=2, fbufs=4)


def _get_compiled(mask: np.ndarray):
    key = (mask.tobytes(), MODE)
    hit = _CACHE.get("key") == key
    if not hit:
        batch_of, n, nbar, nch = _plan(mask)
        segp = _plan_seg(mask) if MODE == "seg" else None
        if segp is not None:
            batch_of = segp["batch_of"]
        nc = _build(nbar, nch, seg=segp, **BEST)
        _CACHE.update(
            key=key, nc=nc, batch_of=batch_of, n=n, nbar=nbar, nch=nch,
            seg=segp,
        )
    return _CACHE


def _build_bench(mask: np.ndarray, reps: int, loop_n: int):
    """Bench-mode program with the same plan/knobs as the real kernel."""
    batch_of, n, nbar, nch = _plan(mask)
    segp = _plan_seg(mask) if MODE == "seg" else None
    return _build(
        nbar, nch, reps=reps, bench_mode=True, loop_n=loop_n, seg=segp,
        **BEST,
    )


def _make_in_maps(h, att_feats, mask, p_att_feats, W_ah, w_alpha,
                  batch_of, n, nbar, nch, seg=None):
    if seg is not None:
        LS = seg["LS"]
        vcnt, n16, nchk, segl = seg["vcnt"], seg["n16"], seg["nchk"], seg["segs"]
        soff_seg = {}
        off = 0
        for j in range(BL):
            for L in LS:
                soff_seg[(j, L)] = off
                off += int(n16[L][j]) // 16
        stot = off
        ncol_j = [int(sum(int(nchk[L][j]) * L for L in LS)) for j in range(BL)]
        boff = np.cumsum([0] + ncol_j)
        tch = int(boff[-1])
    else:
        stot = int(sum(int(v) // 16 for v in nbar))
        tch = int(sum(nch))
        soff = np.cumsum([0] + [int(v) // 16 for v in nbar])
        boff = np.cumsum([0] + list(nch))

    import ml_dtypes

    feats_np = {
        "bf16": ml_dtypes.bfloat16, "f32r": np.float32, "f32": np.float32
    }[ATT_DT]
    p_np = ml_dtypes.bfloat16 if P_DT == "bf16" else np.float32
    ones = np.ones((1, 128), dtype=np.float32)
    oh = np.zeros((BL, BL * 128), dtype=np.float32)
    for j in range(BL):
        oh[j, j * 128 : (j + 1) * 128] = 1.0
    wa_row = np.ascontiguousarray(w_alpha.reshape(1, H))
    # wt[p, rc, hh] = W_ah[hh, rc*128+p]
    wt_arr = np.ascontiguousarray(
        W_ah.T.reshape(RNN // 128, 128, H).transpose(1, 0, 2)
    )

    in_maps = []
    for c in range(NCORES):
        bids = batch_of[c]
        if seg is not None:
            idx_arr = np.full((128, stot), -1, dtype=np.int16)
            bias_arr = np.full((128, tch), NEG, dtype=np.float32)
            for j in range(BL):
                b = int(bids[j])
                col = int(boff[j])
                for L in LS:
                    starts = segl[b][L]
                    cntb = len(starts)
                    v, m16 = int(vcnt[L][j]), int(n16[L][j])
                    assert cntb <= v
                    if m16 == 0:
                        continue
                    arr = np.full(m16, -1, dtype=np.int64)
                    arr[:cntb] = j * M + np.asarray(starts, dtype=np.int64)
                    arr[cntb:v] = j * M  # row-0 pads (fetched, masked out)
                    blk = arr.reshape(m16 // 16, 16).T.astype(np.int16)
                    s0 = soff_seg[(j, L)]
                    idx_arr[:, s0 : s0 + m16 // 16] = np.tile(blk, (8, 1))
                    for cc in range(int(nchk[L][j])):
                        nvalid = min(128, max(0, cntb - cc * 128))
                        for k in range(L):
                            bias_arr[:nvalid, col] = 0.0
                            col += 1
                assert col == int(boff[j + 1])
        else:
            idx_arr = np.zeros((128, stot), dtype=np.int16)
            bias_arr = np.full((128, tch), NEG, dtype=np.float32)
            for j in range(BL):
                b = int(bids[j])
                nb = int(n[b])
                nj = int(nbar[j])
                rows = np.nonzero(mask[b])[0].astype(np.int64)
                assert rows.size == nb
                pad = np.zeros(nj, dtype=np.int64)
                pad[:nb] = rows + j * M
                blk = pad.reshape(nj // 16, 16).T.astype(np.int16)  # [16, nj/16]
                idx_arr[:, int(soff[j]) : int(soff[j + 1])] = np.tile(blk, (8, 1))
                # bias: 0 for valid rows (i < nb), -1e9 otherwise
                for ci in range(nch[j]):
                    i0 = ci * 128
                    nvalid = min(128, max(0, nb - i0))
                    bias_arr[:nvalid, int(boff[j]) + ci] = 0.0
        h_l = h[bids]  # [BL, RNN]
        ht_arr = np.ascontiguousarray(
            h_l.T.reshape(RNN // 128, 128, BL).transpose(1, 0, 2)
        )
        in_maps.append(
            {
                "comb": np.concatenate(
                    [
                        p_att_feats[bids].reshape(BL * M, H).astype(p_np),
                        att_feats[bids].reshape(BL * M, D).astype(feats_np),
                    ],
                    axis=1,
                ),
                "wt": wt_arr,
                "ht": ht_arr,
                "walpha": wa_row,
                "idx": idx_arr,
                "bias": bias_arr,
                "ones": ones,
                "oh": oh,
            }
        )
    return in_maps


def kernel(h, att_feats, att_mask, p_att_feats, W_ah, w_alpha):
    h = np.ascontiguousarray(np.asarray(h, dtype=np.float32))
    att_feats = np.ascontiguousarray(np.asarray(att_feats, dtype=np.float32))
    mask = np.asarray(att_mask).astype(np.int32)
    p_att_feats = np.ascontiguousarray(np.asarray(p_att_feats, dtype=np.float32))
    W_ah = np.ascontiguousarray(np.asarray(W_ah, dtype=np.float32))
    w_alpha = np.ascontiguousarray(np.asarray(w_alpha, dtype=np.float32))

    st = _get_compiled(mask)
    nc, batch_of, n, nbar, nch = st["nc"], st["batch_of"], st["n"], st["nbar"], st["nch"]
    in_maps = _make_in_maps(
        h, att_feats, mask, p_att_feats, W_ah, w_alpha, batch_of, n, nbar,
        nch, seg=st["seg"]
    )

    res = run_bass_kernel_spmd(nc, in_maps, core_ids=list(range(NCORES)))
    kernel._last_results = res  # for test harness introspection

    out = np.empty((B, D), dtype=np.float32)
    for c in range(NCORES):
        o = res.results[c]["out"]
        for j in range(BL):
            out[int(batch_of[c, j])] = o[j]
    return out

